# revision 10
# baseline (speedup 1.0000x reference)
"""Trainium2 Bass kernel for trilinear distance-transform lookup (DT loss).

reference:
  pts = (pc1 + flow)[0]                       # [N,3]
  s   = clip((pts - grid_min) * gf, 0, dim-1) # per-axis
  trilinear gather from dist_grid [H,W,D]
  returns (dist.mean(), dist)

Strategy (8 NeuronCores, data-parallel over points):
  - shard N=1M points into 8 x 125056 slots (last 448 slots of core 7 are
    replicas of the final point; host subtracts their contribution),
    laid out [3, 128, 977] per core (component-major planes).
  - dist_grid (127 MB) replicated in each core's DRAM (padded by 2 zero
    columns so spans can safely over-read).
  - phase A (DVE): clamped coords, floors via int-cast+fixup, weights,
    and the flat element offset of (x0, y0, ze) where ze = 2*floor(sz/2).
    The even ze keeps every gather source 8-byte aligned (odd 4B-aligned
    span starts misread deterministically on HW).
  - gather: per point TWO indirect-DMA descriptors, each copying the
    272 B span grid[x, y0, ze : +68] which contains the three z samples
    (ze, ze+1, ze+2) for both y rows at static offsets {0,1,2, 64,65,66};
    x in {x0, x1}. z-interp is the exact piecewise-linear form
    val = g0 + (g1-g0)*t1 + (g2-g1)*t2, t1 = min(wz', 1),
    t2 = max(wz'-1, 0), wz' = sz - ze.
    SWDGE handles 128 spans (one per partition) per instruction; the
    ~1.3 us/instruction Q7 descriptor generation is the kernel bottleneck.
  - interp (DVE) on strided views of the gathered spans; per-core partial
    sum reduced on-chip; host combines shards.
"""

import sys

sys.path.insert(0, "/opt/trn_rl_repo")

import numpy as np

import concourse.bacc as bacc
import concourse.bass as bass
import concourse.mybir as mybir
import concourse.tile as tile

F32 = mybir.dt.float32
I32 = mybir.dt.int32
OP = mybir.AluOpType

# Problem geometry (fixed by the task spec).
H, W, D = 704, 704, 64
N = 1_000_000
N_CORES = 8
P = 128          # SBUF partitions
F = 977          # slots per partition per core (P*F*N_CORES = 1,000,448)
NPC = P * F      # points (slots) per core
SPAN = 68        # elements fetched per span (68*4 = 272 B, 8B-aligned src)
SLOT = 72        # dest slot elements (288 B, keeps every dest 32B-aligned)
CHUNK = 64       # slots per pipeline chunk
GPAD = 2         # zero pad columns appended to the grid


def build_nc(grid_min, gf, geom=(H, W, D), p=P, f=F, chunk=CHUNK):
    h, w, d = geom
    nc = bacc.Bacc("TRN2", target_bir_lowering=False, debug=False,
                   num_devices=N_CORES)

    pc1 = nc.dram_tensor("pc1", [3, p, f], F32, kind="ExternalInput")
    flow = nc.dram_tensor("flow", [3, p, f], F32, kind="ExternalInput")
    # grid arrives flattened to columns with GPAD zero columns appended
    grid = nc.dram_tensor("grid", [h * w + GPAD, d], F32,
                          kind="ExternalInput")
    dist = nc.dram_tensor("dist", [p, f], F32, kind="ExternalOutput")
    psum = nc.dram_tensor("psum", [p, 1], F32, kind="ExternalOutput")

    # axis=1 => coef==1: offset values are flat element indices.
    grid_flat = grid.ap()
    dims = (float(h - 1), float(w - 1), float(d - 1))

    with tile.TileContext(nc) as tc:
        with (
            tc.tile_pool(name="io", bufs=1) as io,
            tc.tile_pool(name="wk", bufs=1) as wk,
            tc.tile_pool(name="gp", bufs=2) as gp,
            tc.tile_pool(name="ip", bufs=3) as ip,
        ):
            # ---- load inputs ----
            planes = []
            for ci, nm in enumerate("xyz"):
                pt = io.tile([p, f], F32, name=f"p{nm}")
                nc.sync.dma_start(out=pt[:], in_=pc1.ap()[ci])
                fl = io.tile([p, f], F32, name=f"f{nm}")
                nc.sync.dma_start(out=fl[:], in_=flow.ap()[ci])
                planes.append((pt, fl))

            def clamped_coord(ci, scale):
                """s_ci = clip((pc1+flow)*gf, 0, dim-1) * scale"""
                pt, fl = planes[ci]
                s = wk.tile([p, f], F32, name=f"s{ci}")
                nc.vector.tensor_add(s[:], pt[:], fl[:])
                nc.vector.tensor_scalar(
                    s[:], s[:], -float(grid_min[ci]), float(gf),
                    op0=OP.add, op1=OP.mult)
                nc.vector.tensor_scalar(
                    s[:], s[:], 0.0, dims[ci], op0=OP.max, op1=OP.min)
                if scale != 1.0:
                    sh = wk.tile([p, f], F32, name=f"sh{ci}")
                    nc.vector.tensor_scalar(sh[:], s[:], scale, None,
                                            op0=OP.mult)
                    return s, sh
                return s, s

            def floor_tile(src, ci):
                """floor via i32 cast + fixup (valid for RTN or trunc)."""
                ti = wk.tile([p, f], I32, name=f"ti{ci}", tag="ti", bufs=2)
                nc.vector.tensor_copy(out=ti[:], in_=src[:])
                tf = wk.tile([p, f], F32, name=f"tf{ci}", tag="tf", bufs=2)
                nc.vector.tensor_copy(out=tf[:], in_=ti[:])
                g = wk.tile([p, f], F32, name=f"g{ci}", tag="g", bufs=2)
                nc.vector.tensor_tensor(g[:], tf[:], src[:], op=OP.is_gt)
                fl = wk.tile([p, f], F32, name=f"fl{ci}")
                nc.vector.tensor_tensor(fl[:], tf[:], g[:], op=OP.subtract)
                return fl

            # ---- phase A ----
            # x, y: start = min(floor(s), dim-2); w = s - start
            starts, weights = [], []
            for ci in range(2):
                s, _ = clamped_coord(ci, 1.0)
                st = floor_tile(s, ci)
                nc.vector.tensor_scalar(st[:], st[:], dims[ci] - 1.0, None,
                                        op0=OP.min)
                wt = wk.tile([p, f], F32, name=f"wt{ci}")
                nc.vector.tensor_tensor(wt[:], s[:], st[:], op=OP.subtract)
                starts.append(st)
                weights.append(wt)
            # z: ze = 2*floor(s/2) (even, 8B-aligned spans);
            # wz' = s - ze in [0,2);  t1 = min(wz',1);  t2 = max(wz'-1,0)
            sz, szh = clamped_coord(2, 0.5)
            zh = floor_tile(szh, 2)
            zef = wk.tile([p, f], F32, name="zef")
            nc.vector.tensor_scalar(zef[:], zh[:], 2.0, None, op0=OP.mult)
            wzp = wk.tile([p, f], F32, name="wzp")
            nc.vector.tensor_tensor(wzp[:], sz[:], zef[:], op=OP.subtract)
            t1 = wk.tile([p, f], F32, name="t1")
            nc.vector.tensor_scalar(t1[:], wzp[:], 1.0, None, op0=OP.min)
            t2 = wk.tile([p, f], F32, name="t2")
            nc.vector.tensor_scalar(t2[:], wzp[:], -1.0, 0.0,
                                    op0=OP.add, op1=OP.max)

            # flat offset of (x0, y0, ze):  (x0*w + y0)*d + ze
            colf = wk.tile([p, f], F32, name="colf")
            nc.vector.scalar_tensor_tensor(
                colf[:], starts[0][:], float(w), starts[1][:],
                op0=OP.mult, op1=OP.add)
            coli = wk.tile([p, f], I32, name="coli")
            nc.vector.tensor_copy(out=coli[:], in_=colf[:])
            zi = wk.tile([p, f], I32, name="zi")
            nc.vector.tensor_copy(out=zi[:], in_=zef[:])
            shl = int(np.log2(d))
            assert (1 << shl) == d
            nc.vector.tensor_scalar(coli[:], coli[:], shl, None,
                                    op0=OP.logical_shift_left)
            offsA = wk.tile([p, f], I32, name="offsA")
            nc.vector.tensor_tensor(offsA[:], coli[:], zi[:], op=OP.add)
            offsB = wk.tile([p, f], I32, name="offsB")
            nc.vector.tensor_scalar(offsB[:], offsA[:], w * d, None,
                                    op0=OP.add)

            dd_all = io.tile([p, f], F32, name="dd_all")
            wx, wy = weights[0], weights[1]

            # ---- gather + interp, chunked ----
            nch = (f + chunk - 1) // chunk
            for ch in range(nch):
                lo = ch * chunk
                hi = min(lo + chunk, f)
                cs = hi - lo
                sl = slice(lo, hi)
                ga = gp.tile([p, chunk, SLOT], F32, name="ga", tag="ga")
                gb = gp.tile([p, chunk, SLOT], F32, name="gb", tag="gb")
                for j in range(cs):
                    nc.gpsimd.indirect_dma_start(
                        out=ga[:, j, :SPAN], out_offset=None,
                        in_=grid_flat,
                        in_offset=bass.IndirectOffsetOnAxis(
                            ap=offsA[:, lo + j:lo + j + 1], axis=1),
                    )
                    nc.gpsimd.indirect_dma_start(
                        out=gb[:, j, :SPAN], out_offset=None,
                        in_=grid_flat,
                        in_offset=bass.IndirectOffsetOnAxis(
                            ap=offsB[:, lo + j:lo + j + 1], axis=1),
                    )
                # z-interp per y-row: val = g0 + (g1-g0)*t1 + (g2-g1)*t2
                cys = []
                for xi, gt_t in enumerate((ga, gb)):
                    czs = []
                    for yi, base in enumerate((0, d)):
                        g0 = gt_t[:, :cs, base]
                        g1 = gt_t[:, :cs, base + 1]
                        g2 = gt_t[:, :cs, base + 2]
                        t = ip.tile([p, chunk], F32, name="t",
                                    tag=f"t{xi}{yi}")
                        u = ip.tile([p, chunk], F32, name="u",
                                    tag=f"u{xi}{yi}")
                        nc.vector.tensor_tensor(t[:, :cs], g1, g0,
                                                op=OP.subtract)
                        nc.vector.tensor_tensor(t[:, :cs], t[:, :cs],
                                                t1[:, sl], op=OP.mult)
                        nc.vector.tensor_tensor(t[:, :cs], t[:, :cs], g0,
                                                op=OP.add)
                        nc.vector.tensor_tensor(u[:, :cs], g2, g1,
                                                op=OP.subtract)
                        nc.vector.tensor_tensor(u[:, :cs], u[:, :cs],
                                                t2[:, sl], op=OP.mult)
                        nc.vector.tensor_tensor(t[:, :cs], t[:, :cs],
                                                u[:, :cs], op=OP.add)
                        czs.append(t)
                    cy = ip.tile([p, chunk], F32, name="cy", tag=f"cy{xi}")
                    nc.vector.tensor_tensor(cy[:, :cs], czs[1][:, :cs],
                                            czs[0][:, :cs], op=OP.subtract)
                    nc.vector.tensor_tensor(cy[:, :cs], cy[:, :cs],
                                            wy[:, sl], op=OP.mult)
                    nc.vector.tensor_tensor(cy[:, :cs], cy[:, :cs],
                                            czs[0][:, :cs], op=OP.add)
                    cys.append(cy)
                nc.vector.tensor_tensor(dd_all[:, sl], cys[1][:, :cs],
                                        cys[0][:, :cs], op=OP.subtract)
                nc.vector.tensor_tensor(dd_all[:, sl], dd_all[:, sl],
                                        wx[:, sl], op=OP.mult)
                nc.vector.tensor_tensor(dd_all[:, sl], dd_all[:, sl],
                                        cys[0][:, :cs], op=OP.add)

            nc.sync.dma_start(out=dist.ap()[:], in_=dd_all[:])
            ps = io.tile([p, 1], F32, name="ps")
            nc.vector.reduce_sum(out=ps[:], in_=dd_all[:],
                                 axis=mybir.AxisListType.X)
            nc.sync.dma_start(out=psum.ap()[:], in_=ps[:])

    nc.compile()
    return nc


_NC_CACHE = {}


def _get_nc(grid_min, gf):
    key = (tuple(np.asarray(grid_min, np.float32).tolist()), float(gf))
    if key not in _NC_CACHE:
        _NC_CACHE[key] = build_nc(key[0], key[1])
    return _NC_CACHE[key]


def pad_grid(grd, d=D):
    cols = grd.reshape(-1, d)
    return np.concatenate(
        [cols, np.zeros((GPAD, d), np.float32)]).astype(np.float32)


def kernel(pc1, flow, dist_grid, grid_min, grid_factor):
    from concourse import bass_utils

    gf = float(np.asarray(grid_factor))
    gmin = np.asarray(grid_min, np.float32)
    nc = _get_nc(gmin, gf)

    pts = np.asarray(pc1, np.float32).reshape(N, 3)
    flw = np.asarray(flow, np.float32).reshape(N, 3)
    grd = pad_grid(np.ascontiguousarray(np.asarray(dist_grid, np.float32)))

    # pad to 8*128*977 slots by replicating the last point
    total = N_CORES * NPC
    pad = total - N
    pts_p = np.concatenate([pts, np.broadcast_to(pts[-1], (pad, 3))])
    flw_p = np.concatenate([flw, np.broadcast_to(flw[-1], (pad, 3))])

    in_maps = []
    for i in range(N_CORES):
        shard = slice(i * NPC, (i + 1) * NPC)
        in_maps.append({
            "pc1": np.ascontiguousarray(
                pts_p[shard].reshape(P, F, 3).transpose(2, 0, 1)),
            "flow": np.ascontiguousarray(
                flw_p[shard].reshape(P, F, 3).transpose(2, 0, 1)),
            "grid": grd,
        })

    res = bass_utils.run_bass_kernel_spmd(nc, in_maps,
                                          core_ids=list(range(N_CORES)))
    global LAST_RESULTS
    LAST_RESULTS = res
    dist_p = np.concatenate([r["dist"].reshape(NPC) for r in res.results])
    total_sum = sum(float(r["psum"].sum(dtype=np.float64))
                    for r in res.results)
    # remove the padded replicas' contribution
    total_sum -= float(dist_p[N:].sum(dtype=np.float64))
    mean = np.float32(total_sum / N)
    return mean, dist_p[:N]


# revision 15
# speedup vs baseline: 1.0003x; 1.0003x over previous
"""Trainium2 Bass kernel for trilinear distance-transform lookup (DT loss).

reference:
  pts = (pc1 + flow)[0]                       # [N,3]
  s   = clip((pts - grid_min) * gf, 0, dim-1) # per-axis
  trilinear gather from dist_grid [H,W,D]
  returns (dist.mean(), dist)

Strategy (8 NeuronCores, data-parallel over points):
  - shard N=1M points into 8 x 125056 slots (last 448 slots of core 7 are
    replicas of the final point; host subtracts their contribution),
    laid out [3, 128, 977] per core (component-major planes).
  - dist_grid (127 MB) replicated in each core's DRAM (padded by 2 zero
    columns so spans can safely over-read).
  - phase A (DVE): clamped coords, floors via int-cast+fixup, weights,
    and the flat element offset of (x0, y0, ze) where ze = 2*floor(sz/2).
    The even ze keeps every gather source 8-byte aligned (odd 4B-aligned
    span starts misread deterministically on HW).
  - gather: per point TWO indirect-DMA descriptors, each copying the
    272 B span grid[x, y0, ze : +68] which contains the three z samples
    (ze, ze+1, ze+2) for both y rows at static offsets {0,1,2, 64,65,66};
    x in {x0, x1}. z-interp is the exact piecewise-linear form
    val = g0 + (g1-g0)*t1 + (g2-g1)*t2, t1 = min(wz', 1),
    t2 = max(wz'-1, 0), wz' = sz - ze.
    SWDGE handles 128 spans (one per partition) per instruction; the
    ~1.3 us/instruction Q7 descriptor generation is the kernel bottleneck.
  - interp (DVE) on strided views of the gathered spans; per-core partial
    sum reduced on-chip; host combines shards.
"""

import sys

sys.path.insert(0, "/opt/trn_rl_repo")

import numpy as np

import concourse.bacc as bacc
import concourse.bass as bass
import concourse.mybir as mybir
import concourse.tile as tile

F32 = mybir.dt.float32
I32 = mybir.dt.int32
OP = mybir.AluOpType

# Problem geometry (fixed by the task spec).
H, W, D = 704, 704, 64
N = 1_000_000
N_CORES = 8
P = 128          # SBUF partitions
F = 977          # slots per partition per core (P*F*N_CORES = 1,000,448)
NPC = P * F      # points (slots) per core
SPAN = 68        # elements fetched per span (68*4 = 272 B, 8B-aligned src)
SLOT = 72        # dest slot elements (288 B, keeps every dest 32B-aligned)
CHUNK = 48       # slots per pipeline chunk
GPAD = 2         # zero pad columns appended to the grid


def build_nc(grid_min, gf, geom=(H, W, D), p=P, f=F, chunk=CHUNK):
    h, w, d = geom
    nc = bacc.Bacc("TRN2", target_bir_lowering=False, debug=False,
                   num_devices=N_CORES, dynamic_dma_scratch_size=32768)

    pc1 = nc.dram_tensor("pc1", [3, p, f], F32, kind="ExternalInput")
    flow = nc.dram_tensor("flow", [3, p, f], F32, kind="ExternalInput")
    # grid arrives flattened to columns with GPAD zero columns appended
    grid = nc.dram_tensor("grid", [h * w + GPAD, d], F32,
                          kind="ExternalInput")
    dist = nc.dram_tensor("dist", [p, f], F32, kind="ExternalOutput")
    psum = nc.dram_tensor("psum", [p, 1], F32, kind="ExternalOutput")

    # axis=1 => coef==1: offset values are flat element indices.
    grid_flat = grid.ap()
    dims = (float(h - 1), float(w - 1), float(d - 1))

    with tile.TileContext(nc) as tc:
        with tc.tile_pool(name="io", bufs=1) as io:
            wk_cm = tc.tile_pool(name="wk", bufs=1)
            wk = wk_cm.__enter__()
            # ---- load inputs ----
            planes = []
            for ci, nm in enumerate("xyz"):
                pt = wk.tile([p, f], F32, name=f"p{nm}")
                nc.sync.dma_start(out=pt[:], in_=pc1.ap()[ci])
                fl = wk.tile([p, f], F32, name=f"f{nm}")
                nc.sync.dma_start(out=fl[:], in_=flow.ap()[ci])
                planes.append((pt, fl))

            def clamped_coord(ci, scale):
                """s_ci = clip((pc1+flow)*gf, 0, dim-1) * scale"""
                pt, fl = planes[ci]
                s = wk.tile([p, f], F32, name=f"s{ci}")
                nc.vector.tensor_add(s[:], pt[:], fl[:])
                nc.vector.tensor_scalar(
                    s[:], s[:], -float(grid_min[ci]), float(gf),
                    op0=OP.add, op1=OP.mult)
                nc.vector.tensor_scalar(
                    s[:], s[:], 0.0, dims[ci], op0=OP.max, op1=OP.min)
                if scale != 1.0:
                    sh = wk.tile([p, f], F32, name=f"sh{ci}")
                    nc.vector.tensor_scalar(sh[:], s[:], scale, None,
                                            op0=OP.mult)
                    return s, sh
                return s, s

            def floor_tile(src, ci):
                """floor via i32 cast + fixup (valid for RTN or trunc)."""
                ti = wk.tile([p, f], I32, name=f"ti{ci}", tag="ti", bufs=2)
                nc.vector.tensor_copy(out=ti[:], in_=src[:])
                tf = wk.tile([p, f], F32, name=f"tf{ci}", tag="tf", bufs=2)
                nc.vector.tensor_copy(out=tf[:], in_=ti[:])
                g = wk.tile([p, f], F32, name=f"g{ci}", tag="g", bufs=2)
                nc.vector.tensor_tensor(g[:], tf[:], src[:], op=OP.is_gt)
                fl = wk.tile([p, f], F32, name=f"fl{ci}")
                nc.vector.tensor_tensor(fl[:], tf[:], g[:], op=OP.subtract)
                return fl

            # ---- phase A ----
            # x, y: start = min(floor(s), dim-2); w = s - start
            starts, weights = [], []
            for ci in range(2):
                s, _ = clamped_coord(ci, 1.0)
                st = floor_tile(s, ci)
                nc.vector.tensor_scalar(st[:], st[:], dims[ci] - 1.0, None,
                                        op0=OP.min)
                wt = io.tile([p, f], F32, name=f"wt{ci}")
                nc.vector.tensor_tensor(wt[:], s[:], st[:], op=OP.subtract)
                starts.append(st)
                weights.append(wt)
            # z: ze = 2*floor(s/2) (even, 8B-aligned spans);
            # wz' = s - ze in [0,2);  t1 = min(wz',1);  t2 = max(wz'-1,0)
            sz, szh = clamped_coord(2, 0.5)
            zh = floor_tile(szh, 2)
            zef = wk.tile([p, f], F32, name="zef")
            nc.vector.tensor_scalar(zef[:], zh[:], 2.0, None, op0=OP.mult)
            wzp = wk.tile([p, f], F32, name="wzp")
            nc.vector.tensor_tensor(wzp[:], sz[:], zef[:], op=OP.subtract)
            t1 = io.tile([p, f], F32, name="t1")
            nc.vector.tensor_scalar(t1[:], wzp[:], 1.0, None, op0=OP.min)
            t2 = io.tile([p, f], F32, name="t2")
            nc.vector.tensor_scalar(t2[:], wzp[:], -1.0, 0.0,
                                    op0=OP.add, op1=OP.max)

            # flat offset of (x0, y0, ze):  (x0*w + y0)*d + ze
            colf = wk.tile([p, f], F32, name="colf")
            nc.vector.scalar_tensor_tensor(
                colf[:], starts[0][:], float(w), starts[1][:],
                op0=OP.mult, op1=OP.add)
            coli = wk.tile([p, f], I32, name="coli")
            nc.vector.tensor_copy(out=coli[:], in_=colf[:])
            zi = wk.tile([p, f], I32, name="zi")
            nc.vector.tensor_copy(out=zi[:], in_=zef[:])
            shl = int(np.log2(d))
            assert (1 << shl) == d
            nc.vector.tensor_scalar(coli[:], coli[:], shl, None,
                                    op0=OP.logical_shift_left)
            offsA = io.tile([p, f], I32, name="offsA")
            nc.vector.tensor_tensor(offsA[:], coli[:], zi[:], op=OP.add)
            offsB = io.tile([p, f], I32, name="offsB")
            nc.vector.tensor_scalar(offsB[:], offsA[:], w * d, None,
                                    op0=OP.add)

            dd_all = io.tile([p, f], F32, name="dd_all")
            wx, wy = weights[0], weights[1]
            wk_cm.__exit__(None, None, None)

            # ---- gather + interp, chunked ----
            gp_cm = tc.tile_pool(name="gp", bufs=3)
            gp = gp_cm.__enter__()
            ip_cm = tc.tile_pool(name="ip", bufs=3)
            ip = ip_cm.__enter__()
            nch = (f + chunk - 1) // chunk
            for ch in range(nch):
                lo = ch * chunk
                hi = min(lo + chunk, f)
                cs = hi - lo
                sl = slice(lo, hi)
                ga = gp.tile([p, chunk, SLOT], F32, name="ga", tag="ga")
                gb = gp.tile([p, chunk, SLOT], F32, name="gb", tag="gb")
                for j in range(cs):
                    nc.gpsimd.indirect_dma_start(
                        out=ga[:, j, :SPAN], out_offset=None,
                        in_=grid_flat,
                        in_offset=bass.IndirectOffsetOnAxis(
                            ap=offsA[:, lo + j:lo + j + 1], axis=1),
                    )
                    nc.gpsimd.indirect_dma_start(
                        out=gb[:, j, :SPAN], out_offset=None,
                        in_=grid_flat,
                        in_offset=bass.IndirectOffsetOnAxis(
                            ap=offsB[:, lo + j:lo + j + 1], axis=1),
                    )
                # z-interp per y-row: val = g0 + (g1-g0)*t1 + (g2-g1)*t2
                cys = []
                for xi, gt_t in enumerate((ga, gb)):
                    czs = []
                    for yi, base in enumerate((0, d)):
                        g0 = gt_t[:, :cs, base]
                        g1 = gt_t[:, :cs, base + 1]
                        g2 = gt_t[:, :cs, base + 2]
                        t = ip.tile([p, chunk], F32, name="t",
                                    tag=f"t{xi}{yi}")
                        u = ip.tile([p, chunk], F32, name="u",
                                    tag=f"u{xi}{yi}")
                        nc.vector.tensor_tensor(t[:, :cs], g1, g0,
                                                op=OP.subtract)
                        nc.vector.tensor_tensor(t[:, :cs], t[:, :cs],
                                                t1[:, sl], op=OP.mult)
                        nc.vector.tensor_tensor(t[:, :cs], t[:, :cs], g0,
                                                op=OP.add)
                        nc.vector.tensor_tensor(u[:, :cs], g2, g1,
                                                op=OP.subtract)
                        nc.vector.tensor_tensor(u[:, :cs], u[:, :cs],
                                                t2[:, sl], op=OP.mult)
                        nc.vector.tensor_tensor(t[:, :cs], t[:, :cs],
                                                u[:, :cs], op=OP.add)
                        czs.append(t)
                    cy = ip.tile([p, chunk], F32, name="cy", tag=f"cy{xi}")
                    nc.vector.tensor_tensor(cy[:, :cs], czs[1][:, :cs],
                                            czs[0][:, :cs], op=OP.subtract)
                    nc.vector.tensor_tensor(cy[:, :cs], cy[:, :cs],
                                            wy[:, sl], op=OP.mult)
                    nc.vector.tensor_tensor(cy[:, :cs], cy[:, :cs],
                                            czs[0][:, :cs], op=OP.add)
                    cys.append(cy)
                nc.vector.tensor_tensor(dd_all[:, sl], cys[1][:, :cs],
                                        cys[0][:, :cs], op=OP.subtract)
                nc.vector.tensor_tensor(dd_all[:, sl], dd_all[:, sl],
                                        wx[:, sl], op=OP.mult)
                nc.vector.tensor_tensor(dd_all[:, sl], dd_all[:, sl],
                                        cys[0][:, :cs], op=OP.add)

            nc.sync.dma_start(out=dist.ap()[:], in_=dd_all[:])
            ps = io.tile([p, 1], F32, name="ps")
            nc.vector.reduce_sum(out=ps[:], in_=dd_all[:],
                                 axis=mybir.AxisListType.X)
            nc.sync.dma_start(out=psum.ap()[:], in_=ps[:])
            ip_cm.__exit__(None, None, None)
            gp_cm.__exit__(None, None, None)

    nc.compile()
    return nc


_NC_CACHE = {}


def _get_nc(grid_min, gf):
    key = (tuple(np.asarray(grid_min, np.float32).tolist()), float(gf))
    if key not in _NC_CACHE:
        _NC_CACHE[key] = build_nc(key[0], key[1])
    return _NC_CACHE[key]


def pad_grid(grd, d=D):
    cols = grd.reshape(-1, d)
    return np.concatenate(
        [cols, np.zeros((GPAD, d), np.float32)]).astype(np.float32)


def kernel(pc1, flow, dist_grid, grid_min, grid_factor):
    from concourse import bass_utils

    gf = float(np.asarray(grid_factor))
    gmin = np.asarray(grid_min, np.float32)
    nc = _get_nc(gmin, gf)

    pts = np.asarray(pc1, np.float32).reshape(N, 3)
    flw = np.asarray(flow, np.float32).reshape(N, 3)
    grd = pad_grid(np.ascontiguousarray(np.asarray(dist_grid, np.float32)))

    # pad to 8*128*977 slots by replicating the last point
    total = N_CORES * NPC
    pad = total - N
    pts_p = np.concatenate([pts, np.broadcast_to(pts[-1], (pad, 3))])
    flw_p = np.concatenate([flw, np.broadcast_to(flw[-1], (pad, 3))])

    in_maps = []
    for i in range(N_CORES):
        shard = slice(i * NPC, (i + 1) * NPC)
        in_maps.append({
            "pc1": np.ascontiguousarray(
                pts_p[shard].reshape(P, F, 3).transpose(2, 0, 1)),
            "flow": np.ascontiguousarray(
                flw_p[shard].reshape(P, F, 3).transpose(2, 0, 1)),
            "grid": grd,
        })

    res = bass_utils.run_bass_kernel_spmd(nc, in_maps,
                                          core_ids=list(range(N_CORES)))
    global LAST_RESULTS
    LAST_RESULTS = res
    dist_p = np.concatenate([r["dist"].reshape(NPC) for r in res.results])
    total_sum = sum(float(r["psum"].sum(dtype=np.float64))
                    for r in res.results)
    # remove the padded replicas' contribution
    total_sum -= float(dist_p[N:].sum(dtype=np.float64))
    mean = np.float32(total_sum / N)
    return mean, dist_p[:N]
